# revision 21
# baseline (speedup 1.0000x reference)
"""Block-circulant process via frequency-domain factorization on 8 cores.

out = x @ M factorizes through the (truncated, 48-bin) real FFT:
  stage A: per in-block j:  S[(p,f), b] = sum_t F[t,(p,f)] xT[jB+t, b]
  stage M: per freq pair e: mid[(q,i), b] = sum_{p,j} W_e[(p,j),(q,i)] S
  stage C: per out-block i: out[t, b] = sum_{q,f} G[(q,f), t] mid

v2: everything bf16 on device (host casts in/out), intermediates stay
in SBUF, and the two partition-regroups are SBUF->SBUF DMAs instead of
DRAM bounces. Batch is split in CH chunks per core so chunk k+1's
stage A overlaps chunk k's regroups/middle stages. HBM per core is
~9 MiB (x in, out, weights) vs ~41 MiB for the DRAM-bounce version.

Sharding: pure data-parallel over batch (x dim 0), weights replicated.
"""

import numpy as np

B = 128
K_HALF = B // 2 + 1  # 65
KT = 48  # frequency truncation
KI = 32
KO = 32
BATCH = 4096
IN_F = 4096
OUT_F = 4096

N_CORES = 8
BQ = BATCH // N_CORES  # 512 batch rows per core
NP = KT // 2  # 24 frequency pairs

CH = 2  # batch chunks per core (pipeline depth)
BC = BQ // CH  # 256 batch columns per chunk
JG = 8  # j-blocks per x-load DMA / i-blocks per out-store DMA

_CACHE = {}
LAST_RESULTS = None
TRACE = False


def _build_nc():
    import concourse.bacc as bacc
    import concourse.mybir as mybir
    import concourse.tile as tile

    BF16 = mybir.dt.bfloat16
    F32 = mybir.dt.float32

    nc = bacc.Bacc(None, target_bir_lowering=False)
    # x packed on host: [t, (c, j, b)]
    xP = nc.declare_dram_parameter("xP", [128, CH * KI * BC], BF16,
                                   isOutput=False)
    fmat = nc.declare_dram_parameter("fmat", [128, 96], BF16, isOutput=False)
    gmat = nc.declare_dram_parameter("gmat", [96, 128], BF16, isOutput=False)
    wmid = nc.declare_dram_parameter("wmid", [128, NP * 128], BF16,
                                     isOutput=False)
    # out packed: [t, (c, i, b)]; host unpacks + upcasts
    oP = nc.declare_dram_parameter("oP", [128, CH * KO * BC], BF16,
                                   isOutput=True)

    with tile.TileContext(nc) as tc:
        with (
            tc.tile_pool(name="cpool", bufs=1) as cpool,
            tc.tile_pool(name="xpool", bufs=2 * (KI // JG) + 2) as xpool,
            tc.tile_pool(name="spool", bufs=2) as spool,
            tc.tile_pool(name="gpool", bufs=NP + 8) as gpool,
            tc.tile_pool(name="mpool", bufs=2) as mpool,
            tc.tile_pool(name="hpool", bufs=KO + 8) as hpool,
            tc.tile_pool(name="opool", bufs=2 * (KO // JG) + 2) as opool,
            tc.tile_pool(name="psA", bufs=2, space="PSUM") as psA,
            tc.tile_pool(name="psM", bufs=2, space="PSUM") as psM,
            tc.tile_pool(name="psC", bufs=3, space="PSUM") as psC,
        ):
            f_t = cpool.tile([128, 96], BF16, name="f_t")
            nc.scalar.dma_start(f_t[:], fmat[:])
            g_t = cpool.tile([96, 128], BF16, name="g_t")
            nc.scalar.dma_start(g_t[:], gmat[:])
            w_all = cpool.tile([128, NP * 128], BF16, name="w_all")
            nc.scalar.dma_start(w_all[:], wmid[:])

            # One writer ENGINE per destination tile: Tile's cross-engine
            # subtile dep tracking drops waits when a tile has writers on
            # multiple engines, so keep each tile single-engine.
            def cp(eng, dst, src):
                if eng is nc.scalar:
                    eng.copy(dst, src)
                else:
                    eng.tensor_copy(dst, src)

            for c in range(CH):
                # ---- load x chunk (4 DMAs of [128, JG*BC]) ----
                x_g = []
                for g in range(KI // JG):
                    xt = xpool.tile([128, JG * BC], BF16, name="x_g",
                                    tag="x_g")
                    col0 = (c * KI + g * JG) * BC
                    nc.sync.dma_start(xt[:], xP[:, col0:col0 + JG * BC])
                    x_g.append(xt)

                # ---- stage A: j-pair matmuls -> S_c [96, (j b)] bf16 ----
                s_c = spool.tile([96, KI * BC], BF16, name="s_c", tag="s_c")
                JP = 512 // BC  # j's per matmul (free dim 512)
                for j2 in range(KI // JP):
                    ps = psA.tile([96, JP * BC], mybir.dt.float32,
                                  name="ps_a", tag="ps_a")
                    g = (j2 * JP) // JG
                    off = (j2 * JP) % JG
                    rhs = x_g[g][:, off * BC:(off + JP) * BC]
                    nc.tensor.matmul(ps[:], f_t[:], rhs, start=True,
                                     stop=True)
                    cp(nc.vector, s_c[:, j2 * JP * BC:(j2 + 1) * JP * BC],
                       ps[:])

                # ---- regroup 1: SBUF->SBUF, per freq pair e ----
                # s_c partition order is (e fl p) [host F column order], so
                # the src is a plain contiguous 4-partition slice; the dst
                # is one whole tile with partition order (fl p j).
                # DMA lowering only handles plain rectangular SBUF dst APs,
                # so scatter row (e, flp) of S onto the 32-partition block
                # (flp, j=0..31) of g_e with one 2D DMA per (e, flp).
                g_tiles = []
                for e in range(NP):
                    gt = gpool.tile([128, BC], BF16, name="g_e", tag="g_e")
                    for flp in range(4):
                        r = e * 4 + flp
                        nc.sync.dma_start(gt[flp * 32:(flp + 1) * 32, :],
                                          s_c[r:r + 1, :])
                    g_tiles.append(gt)

                # ---- stage M: 24 matmuls -> mid_c [(fl q i), (e b)] ----
                mid_c = mpool.tile([128, NP * BC], BF16, name="mid_c",
                                   tag="mid_c")
                for e in range(NP):
                    ps = psM.tile([128, BC], mybir.dt.float32, name="ps_m",
                                  tag="ps_m")
                    nc.tensor.matmul(ps[:], w_all[:, e * 128:(e + 1) * 128],
                                     g_tiles[e][:], start=True, stop=True)
                    cp(nc.scalar, mid_c[:, e * BC:(e + 1) * BC], ps[:])

                # ---- regroup 2: SBUF->SBUF, per out-block i ----
                # mid_c partition order is (i fl q) [host W column order],
                # so the src is a plain contiguous 4-partition slice; the
                # dst has partition order (fl q e) [host G row order].
                IP = 512 // BC  # i's per stage-C matmul
                h_tiles = []
                for i in range(KO):
                    if i % IP == 0:
                        ht = hpool.tile([96, IP * BC], BF16, name="h_i",
                                        tag="h_i")
                        h_tiles.append(ht)
                    for flq in range(4):
                        r = i * 4 + flq
                        nc.scalar.dma_start(
                            h_tiles[-1][flq * 24:(flq + 1) * 24,
                                        (i % IP) * BC:(i % IP + 1) * BC],
                            mid_c[r:r + 1, :])

                # ---- stage C: i-pair matmuls -> out tiles, group store ----
                o_g = None
                o_eng = nc.vector
                for i2 in range(KO // IP):
                    if (i2 * IP) % JG == 0:
                        o_g = opool.tile([128, JG * BC], BF16, name="o_g",
                                         tag="o_g")
                        o_eng = nc.vector if (i2 * IP // JG) % 2 == 0 \
                            else nc.scalar
                    ps = psC.tile([128, IP * BC], mybir.dt.float32,
                                  name="ps_c", tag="ps_c")
                    nc.tensor.matmul(ps[:], g_t[:], h_tiles[i2][:],
                                     start=True, stop=True)
                    off = (i2 * IP) % JG
                    cp(o_eng, o_g[:, off * BC:(off + IP) * BC], ps[:])
                    if ((i2 + 1) * IP) % JG == 0:
                        col0 = (c * KO + (i2 + 1) * IP - JG) * BC
                        nc.sync.dma_start(oP[:, col0:col0 + JG * BC], o_g[:])
    nc.finalize()
    return nc


def _get_nc():
    if "nc" not in _CACHE:
        _CACHE["nc"] = _build_nc()
    return _CACHE["nc"]


def _host_weights(W_real, W_imag):
    """F [128,96], G [96,128], Wmid [24,128,128] (float64 -> caller casts)."""
    t = np.arange(B)[:, None].astype(np.float64)
    # F columns ordered (e, fl, p): f = 2e + fl; p=0 -> cos, p=1 -> -sin.
    # (e, fl, p) order makes regroup-1's source a contiguous 4-partition
    # slice of S per e.
    F = np.zeros((128, 96))
    for e in range(NP):
        for fl in range(2):
            for p in range(2):
                f = 2 * e + fl
                col = e * 4 + fl * 2 + p
                w = 2 * np.pi * f * t[:, 0] / B
                F[:, col] = np.cos(w) if p == 0 else -np.sin(w)
    # G rows ordered (fl, q, e) with f = 2e + fl, matching the H-tile
    # partition layout: q=0 -> scale*cos, q=1 -> -scale*sin
    G = np.zeros((96, 128))
    scale = np.full(KT, 2.0 / B)
    scale[0] = 1.0 / B
    for fl in range(2):
        for q in range(2):
            for e in range(NP):
                f = 2 * e + fl
                w = 2 * np.pi * f * np.arange(B) / B
                G[fl * 48 + q * 24 + e] = (scale[f] * np.cos(w) if q == 0
                                           else -scale[f] * np.sin(w))
    # Wmid[e]: rows (fl, p, j), cols (i, fl, q); block-diag in fl.
    # Build with (fl, q, i) column slices first, then permute columns to
    # (i, fl, q) -- that order makes regroup-2's source a contiguous
    # 4-partition slice of mid per i.
    Wr = W_real.astype(np.float64)
    Wi = W_imag.astype(np.float64)
    Wm = np.zeros((NP, 128, 128))
    for e in range(NP):
        for fl in range(2):
            f = 2 * e + fl
            r0, c0 = fl * 64, fl * 64
            # q=0: Re_out = Wr @ Re + Wi @ Im ; q=1: Im_out = Wr @ Im - Wi @ Re
            Wrf = Wr[:, :, f].T  # [j, i]
            Wif = Wi[:, :, f].T
            Wm[e, r0:r0 + 32, c0:c0 + 32] = Wrf            # p0 -> q0: Wr
            Wm[e, r0 + 32:r0 + 64, c0:c0 + 32] = Wif       # p1 -> q0: Wi
            Wm[e, r0:r0 + 32, c0 + 32:c0 + 64] = -Wif      # p0 -> q1: -Wi
            Wm[e, r0 + 32:r0 + 64, c0 + 32:c0 + 64] = Wrf  # p1 -> q1: Wr
    # column permutation: new col (i*4 + fl*2 + q) <- old col (fl*64+q*32+i)
    idx = np.empty(128, np.int64)
    for i in range(32):
        for fl in range(2):
            for q in range(2):
                idx[i * 4 + fl * 2 + q] = fl * 64 + q * 32 + i
    Wm = Wm[:, :, idx]
    return F, G, Wm


def kernel(x, W_real, W_imag):
    global LAST_RESULTS
    import ml_dtypes
    from concourse.bass_utils import run_bass_kernel_spmd

    bf16 = np.dtype(ml_dtypes.bfloat16)
    x = np.asarray(x, dtype=np.float32)
    F, G, Wm = _host_weights(np.asarray(W_real), np.asarray(W_imag))
    Fb = np.ascontiguousarray(F).astype(bf16)
    Gb = np.ascontiguousarray(G).astype(bf16)
    wm_packed = np.ascontiguousarray(
        Wm.transpose(1, 0, 2).reshape(128, NP * 128)).astype(bf16)

    # pack x: [batch, (j t)] -> per core [t, (c, j, b)]
    xr = x.reshape(N_CORES, CH, BC, KI, B).transpose(0, 4, 1, 3, 2)
    xr = np.ascontiguousarray(xr.reshape(N_CORES, B, CH * KI * BC)).astype(
        bf16)

    in_maps = []
    for core in range(N_CORES):
        in_maps.append({"xP": xr[core], "fmat": Fb, "gmat": Gb,
                        "wmid": wm_packed})

    nc = _get_nc()
    res = run_bass_kernel_spmd(nc, in_maps, list(range(N_CORES)), trace=TRACE)
    LAST_RESULTS = res

    out = np.empty((BATCH, OUT_F), np.float32)
    for core in range(N_CORES):
        oPc = np.asarray(res.results[core]["oP"]).astype(np.float32)
        # [t, (c, i, b)] -> [(c b), (i t)]
        oc = oPc.reshape(B, CH, KO, BC).transpose(1, 3, 2, 0)
        out[core * BQ:(core + 1) * BQ, :] = oc.reshape(BQ, OUT_F)
    return out


# revision 23
# speedup vs baseline: 2.8997x; 2.8997x over previous
"""Block-circulant process via frequency-domain factorization on 8 cores.

out = x @ M factorizes through the (truncated, 48-bin) real FFT:
  stage A: per in-block j:  S[(p,f), b] = sum_t F[t,(p,f)] xT[jB+t, b]
  stage M: per freq pair e: mid[(q,i), b] = sum_{p,j} W_e[(p,j),(q,i)] S
  stage C: per out-block i: out[t, b] = sum_{q,f} G[(q,f), t] mid

v2: everything bf16 on device (host casts in/out), intermediates stay
in SBUF, and the two partition-regroups are SBUF->SBUF DMAs instead of
DRAM bounces. Batch is split in CH chunks per core so chunk k+1's
stage A overlaps chunk k's regroups/middle stages. HBM per core is
~9 MiB (x in, out, weights) vs ~41 MiB for the DRAM-bounce version.

Sharding: pure data-parallel over batch (x dim 0), weights replicated.
"""

import numpy as np

B = 128
K_HALF = B // 2 + 1  # 65
KT = 48  # frequency truncation
KI = 32
KO = 32
BATCH = 4096
IN_F = 4096
OUT_F = 4096

N_CORES = 8
BQ = BATCH // N_CORES  # 512 batch rows per core
NP = KT // 2  # 24 frequency pairs

CH = 2  # batch chunks per core (pipeline depth)
BC = BQ // CH  # 256 batch columns per chunk
JG = 8  # j-blocks per x-load DMA / i-blocks per out-store DMA

_CACHE = {}
LAST_RESULTS = None
TRACE = False


def _build_nc():
    import concourse.bacc as bacc
    import concourse.mybir as mybir
    import concourse.tile as tile

    BF16 = mybir.dt.bfloat16
    F32 = mybir.dt.float32

    nc = bacc.Bacc(None, target_bir_lowering=False)
    # x packed on host: [t, (c, j, b)]
    xP = nc.declare_dram_parameter("xP", [128, CH * KI * BC], BF16,
                                   isOutput=False)
    fmat = nc.declare_dram_parameter("fmat", [128, 96], BF16, isOutput=False)
    gmat = nc.declare_dram_parameter("gmat", [96, 128], BF16, isOutput=False)
    wmid = nc.declare_dram_parameter("wmid", [128, NP * 128], BF16,
                                     isOutput=False)
    # out packed: [t, (c, i, b)]; host unpacks + upcasts
    oP = nc.declare_dram_parameter("oP", [128, CH * KO * BC], BF16,
                                   isOutput=True)

    # DRAM scratch for the two partition-regroups: the write DMAs apply
    # the permutation (DRAM dst APs may be 3-dim), the read DMAs are then
    # plain column-slices with 2KB descriptors.
    # sS_g[(flp j), (c e b)] ; sM_h[(flq e), (c i b)]
    sS_g = nc.dram_tensor("sS_g", [128, CH * NP * BC], BF16)
    sM_h = nc.dram_tensor("sM_h", [96, CH * KO * BC], BF16)
    sS_w = sS_g.rearrange("(flp j) (c e b) -> c flp e j b", flp=4, j=KI,
                          c=CH, e=NP)
    sM_w = sM_h.rearrange("(flq e) (c i b) -> c flq i e b", flq=4, e=NP,
                          c=CH, i=KO)

    with tile.TileContext(nc) as tc:
        with (
            tc.tile_pool(name="cpool", bufs=1) as cpool,
            tc.tile_pool(name="xpool", bufs=2 * (KI // JG) + 2) as xpool,
            tc.tile_pool(name="spool", bufs=2) as spool,
            tc.tile_pool(name="gpool", bufs=NP // 4 + 3) as gpool,
            tc.tile_pool(name="mpool", bufs=2) as mpool,
            tc.tile_pool(name="hpool", bufs=KO // 4 + 3) as hpool,
            tc.tile_pool(name="opool", bufs=2 * (KO // JG) + 2) as opool,
            tc.tile_pool(name="psA", bufs=2, space="PSUM") as psA,
            tc.tile_pool(name="psM", bufs=2, space="PSUM") as psM,
            tc.tile_pool(name="psC", bufs=3, space="PSUM") as psC,
        ):
            f_t = cpool.tile([128, 96], BF16, name="f_t")
            nc.scalar.dma_start(f_t[:], fmat[:])
            g_t = cpool.tile([96, 128], BF16, name="g_t")
            nc.scalar.dma_start(g_t[:], gmat[:])
            w_all = cpool.tile([128, NP * 128], BF16, name="w_all")
            nc.scalar.dma_start(w_all[:], wmid[:])

            # One writer ENGINE per destination tile: Tile's cross-engine
            # subtile dep tracking drops waits when a tile has writers on
            # multiple engines, so keep each tile single-engine.
            def cp(eng, dst, src):
                if eng is nc.scalar:
                    eng.copy(dst, src)
                else:
                    eng.tensor_copy(dst, src)

            for c in range(CH):
                # ---- load x chunk (4 DMAs of [128, JG*BC]) ----
                x_g = []
                for g in range(KI // JG):
                    xt = xpool.tile([128, JG * BC], BF16, name="x_g",
                                    tag="x_g")
                    col0 = (c * KI + g * JG) * BC
                    nc.sync.dma_start(xt[:], xP[:, col0:col0 + JG * BC])
                    x_g.append(xt)

                # ---- stage A: j-pair matmuls -> S_c [96, (j b)] bf16 ----
                s_c = spool.tile([96, KI * BC], BF16, name="s_c", tag="s_c")
                JP = 512 // BC  # j's per matmul (free dim 512)
                for j2 in range(KI // JP):
                    ps = psA.tile([96, JP * BC], mybir.dt.float32,
                                  name="ps_a", tag="ps_a")
                    g = (j2 * JP) // JG
                    off = (j2 * JP) % JG
                    rhs = x_g[g][:, off * BC:(off + JP) * BC]
                    nc.tensor.matmul(ps[:], f_t[:], rhs, start=True,
                                     stop=True)
                    cp(nc.vector, s_c[:, j2 * JP * BC:(j2 + 1) * JP * BC],
                       ps[:])

                # ---- regroup 1 via DRAM: 4 permuting writes, then
                # quad-of-e reads as plain column slices ----
                for flp in range(4):
                    nc.sync.dma_start(
                        sS_w[c, flp],
                        s_c[flp * 24:(flp + 1) * 24, :].rearrange(
                            "e (j b) -> e j b", j=KI))
                EQ = 4  # e's per read DMA
                g_quads = []
                for qd in range(NP // EQ):
                    gq = gpool.tile([128, EQ * BC], BF16, name="g_q",
                                    tag="g_q")
                    col0 = (c * NP + qd * EQ) * BC
                    nc.sync.dma_start(gq[:], sS_g[:, col0:col0 + EQ * BC])
                    g_quads.append(gq)

                # ---- stage M: 24 matmuls -> mid_c [(i fl q), (e b)] ----
                mid_c = mpool.tile([128, NP * BC], BF16, name="mid_c",
                                   tag="mid_c")
                for e in range(NP):
                    ps = psM.tile([128, BC], mybir.dt.float32, name="ps_m",
                                  tag="ps_m")
                    rhs = g_quads[e // EQ][:, (e % EQ) * BC:(e % EQ + 1) * BC]
                    nc.tensor.matmul(ps[:], w_all[:, e * 128:(e + 1) * 128],
                                     rhs, start=True, stop=True)
                    cp(nc.scalar, mid_c[:, e * BC:(e + 1) * BC], ps[:])

                # ---- regroup 2 via DRAM: 4 permuting writes, then
                # quad-of-i reads as plain column slices ----
                for flq in range(4):
                    nc.scalar.dma_start(
                        sM_w[c, flq],
                        mid_c[flq * 32:(flq + 1) * 32, :].rearrange(
                            "i (e b) -> i e b", e=NP))
                IP = 512 // BC  # i's per stage-C matmul
                IQ = 4  # i's per read DMA
                h_quads = []
                for hq in range(KO // IQ):
                    ht = hpool.tile([96, IQ * BC], BF16, name="h_q",
                                    tag="h_q")
                    col0 = (c * KO + hq * IQ) * BC
                    nc.scalar.dma_start(ht[:], sM_h[:, col0:col0 + IQ * BC])
                    h_quads.append(ht)

                # ---- stage C: i-pair matmuls -> out tiles, group store ----
                o_g = None
                o_eng = nc.vector
                for i2 in range(KO // IP):
                    if (i2 * IP) % JG == 0:
                        o_g = opool.tile([128, JG * BC], BF16, name="o_g",
                                         tag="o_g")
                        o_eng = nc.vector if (i2 * IP // JG) % 2 == 0 \
                            else nc.scalar
                    ps = psC.tile([128, IP * BC], mybir.dt.float32,
                                  name="ps_c", tag="ps_c")
                    rhs = h_quads[(i2 * IP) // IQ][
                        :, (i2 * IP % IQ) * BC:(i2 * IP % IQ + IP) * BC]
                    nc.tensor.matmul(ps[:], g_t[:], rhs,
                                     start=True, stop=True)
                    off = (i2 * IP) % JG
                    cp(o_eng, o_g[:, off * BC:(off + IP) * BC], ps[:])
                    if ((i2 + 1) * IP) % JG == 0:
                        col0 = (c * KO + (i2 + 1) * IP - JG) * BC
                        nc.sync.dma_start(oP[:, col0:col0 + JG * BC], o_g[:])
    nc.finalize()
    return nc


def _get_nc():
    if "nc" not in _CACHE:
        _CACHE["nc"] = _build_nc()
    return _CACHE["nc"]


def _host_weights(W_real, W_imag):
    """F [128,96], G [96,128], Wmid [24,128,128] (float64 -> caller casts)."""
    t = np.arange(B)[:, None].astype(np.float64)
    # F columns ordered (fl, p, e): f = 2e + fl; p=0 -> cos, p=1 -> -sin.
    # (fl p, e) order makes regroup-1's write source a contiguous
    # 24-partition slice of S per (fl, p).
    F = np.zeros((128, 96))
    for fl in range(2):
        for p in range(2):
            for e in range(NP):
                f = 2 * e + fl
                col = fl * 48 + p * 24 + e
                w = 2 * np.pi * f * t[:, 0] / B
                F[:, col] = np.cos(w) if p == 0 else -np.sin(w)
    # G rows ordered (fl, q, e) with f = 2e + fl, matching the H-tile
    # partition layout: q=0 -> scale*cos, q=1 -> -scale*sin
    G = np.zeros((96, 128))
    scale = np.full(KT, 2.0 / B)
    scale[0] = 1.0 / B
    for fl in range(2):
        for q in range(2):
            for e in range(NP):
                f = 2 * e + fl
                w = 2 * np.pi * f * np.arange(B) / B
                G[fl * 48 + q * 24 + e] = (scale[f] * np.cos(w) if q == 0
                                           else -scale[f] * np.sin(w))
    # Wmid[e]: rows (fl, p, j), cols (fl, q, i); block-diag in fl.
    # (fl q, i) column order makes regroup-2's write source a contiguous
    # 32-partition slice of mid per (fl, q).
    Wr = W_real.astype(np.float64)
    Wi = W_imag.astype(np.float64)
    Wm = np.zeros((NP, 128, 128))
    for e in range(NP):
        for fl in range(2):
            f = 2 * e + fl
            r0, c0 = fl * 64, fl * 64
            # q=0: Re_out = Wr @ Re + Wi @ Im ; q=1: Im_out = Wr @ Im - Wi @ Re
            Wrf = Wr[:, :, f].T  # [j, i]
            Wif = Wi[:, :, f].T
            Wm[e, r0:r0 + 32, c0:c0 + 32] = Wrf            # p0 -> q0: Wr
            Wm[e, r0 + 32:r0 + 64, c0:c0 + 32] = Wif       # p1 -> q0: Wi
            Wm[e, r0:r0 + 32, c0 + 32:c0 + 64] = -Wif      # p0 -> q1: -Wi
            Wm[e, r0 + 32:r0 + 64, c0 + 32:c0 + 64] = Wrf  # p1 -> q1: Wr
    return F, G, Wm


def kernel(x, W_real, W_imag):
    global LAST_RESULTS
    import ml_dtypes
    from concourse.bass_utils import run_bass_kernel_spmd

    bf16 = np.dtype(ml_dtypes.bfloat16)
    x = np.asarray(x, dtype=np.float32)
    F, G, Wm = _host_weights(np.asarray(W_real), np.asarray(W_imag))
    Fb = np.ascontiguousarray(F).astype(bf16)
    Gb = np.ascontiguousarray(G).astype(bf16)
    wm_packed = np.ascontiguousarray(
        Wm.transpose(1, 0, 2).reshape(128, NP * 128)).astype(bf16)

    # pack x: [batch, (j t)] -> per core [t, (c, j, b)]
    xr = x.reshape(N_CORES, CH, BC, KI, B).transpose(0, 4, 1, 3, 2)
    xr = np.ascontiguousarray(xr.reshape(N_CORES, B, CH * KI * BC)).astype(
        bf16)

    in_maps = []
    for core in range(N_CORES):
        in_maps.append({"xP": xr[core], "fmat": Fb, "gmat": Gb,
                        "wmid": wm_packed})

    nc = _get_nc()
    res = run_bass_kernel_spmd(nc, in_maps, list(range(N_CORES)), trace=TRACE)
    LAST_RESULTS = res

    out = np.empty((BATCH, OUT_F), np.float32)
    for core in range(N_CORES):
        oPc = np.asarray(res.results[core]["oP"]).astype(np.float32)
        # [t, (c, i, b)] -> [(c b), (i t)]
        oc = oPc.reshape(B, CH, KO, BC).transpose(1, 3, 2, 0)
        out[core * BQ:(core + 1) * BQ, :] = oc.reshape(BQ, OUT_F)
    return out


# revision 24
# speedup vs baseline: 3.2537x; 1.1221x over previous
"""Block-circulant process via frequency-domain factorization on 8 cores.

out = x @ M factorizes through the (truncated, 48-bin) real FFT:
  stage A: per in-block j:  S[(fl p e), b] = sum_t F[t,(fl p e)] x[jB+t, b]
  stage M: per freq pair e: mid[(fl q i), b] = W_e[(fl p j),(fl q i)]^T G_e
  stage C: per out-block i: out[t, b] = sum_{fl q e} G[(fl q e), t] H_i

All device data is bf16 (host casts in/out; fp32 accumulation in PSUM).
The two partition-regroups bounce through DRAM: 4+4 permuting writes
(contiguous-partition SBUF sources, 3-dim DRAM dst APs) + a few plain
column-slice reads with 2KB descriptors. Matmuls run in pairs into
2-bank PSUM tiles so one PSUM->SBUF copy serves two matmuls; copies
split across DVE and ACT via disjoint single-writer destination tiles.

Sharding: pure data-parallel over batch (x dim 0), weights replicated.
"""

import numpy as np

B = 128
K_HALF = B // 2 + 1  # 65
KT = 48  # frequency truncation
KI = 32
KO = 32
BATCH = 4096
IN_F = 4096
OUT_F = 4096

N_CORES = 8
BQ = BATCH // N_CORES  # 512 batch columns per core
NP = KT // 2  # 24 frequency pairs

BC = BQ  # single batch chunk
JG = 8   # j-blocks per x-load DMA / i-blocks per out-store DMA
EQ = 6   # e's per regroup-1 read DMA
IQ = 8   # i's per regroup-2 read DMA

_CACHE = {}
LAST_RESULTS = None
TRACE = False


def _build_nc():
    import concourse.bacc as bacc
    import concourse.mybir as mybir
    import concourse.tile as tile

    BF16 = mybir.dt.bfloat16

    nc = bacc.Bacc(None, target_bir_lowering=False)
    # x packed on host: [t, (j, b)]
    xP = nc.declare_dram_parameter("xP", [128, KI * BC], BF16,
                                   isOutput=False)
    fmat = nc.declare_dram_parameter("fmat", [128, 96], BF16, isOutput=False)
    gmat = nc.declare_dram_parameter("gmat", [96, 128], BF16, isOutput=False)
    wmid = nc.declare_dram_parameter("wmid", [128, NP * 128], BF16,
                                     isOutput=False)
    # out packed: [t, (i, b)]; host unpacks + upcasts
    oP = nc.declare_dram_parameter("oP", [128, KO * BC], BF16,
                                   isOutput=True)

    # DRAM scratch for the two partition-regroups.
    # sS_g[(flp j), (e b)] ; sM_h[(flq e), (i b)]
    sS_g = nc.dram_tensor("sS_g", [128, NP * BC], BF16)
    sM_h = nc.dram_tensor("sM_h", [96, KO * BC], BF16)
    # write views
    sS_w = sS_g.rearrange("(flp j) (e b) -> flp e j b", flp=4, j=KI, e=NP)
    sM_w = sM_h.rearrange("(flq e) (i b) -> flq i e b", flq=4, e=NP, i=KO)

    JH = KI // 2  # 16 j's per s half
    EH = NP // 2  # 12 e's per mid half

    with tile.TileContext(nc) as tc:
        with (
            tc.tile_pool(name="cpool", bufs=1) as cpool,
            tc.tile_pool(name="xpool", bufs=KI // JG + 1) as xpool,
            tc.tile_pool(name="spool", bufs=1) as spool,
            tc.tile_pool(name="gpool", bufs=NP // EQ + 1) as gpool,
            tc.tile_pool(name="mpool", bufs=1) as mpool,
            tc.tile_pool(name="hpool", bufs=KO // IQ) as hpool,
            tc.tile_pool(name="opool", bufs=2) as opool,
            tc.tile_pool(name="psum", bufs=4, space="PSUM") as psum,
        ):
            f_t = cpool.tile([128, 96], BF16, name="f_t")
            nc.scalar.dma_start(f_t[:], fmat[:])
            g_t = cpool.tile([96, 128], BF16, name="g_t")
            nc.scalar.dma_start(g_t[:], gmat[:])
            w_all = cpool.tile([128, NP * 128], BF16, name="w_all")
            nc.scalar.dma_start(w_all[:], wmid[:])

            # Tile's cross-engine subtile dep tracking drops waits when a
            # tile has writers on multiple engines -> keep each destination
            # tile single-engine (DVE writes *_lo, ACT writes *_hi).
            # ---- load x in 4 group DMAs ----
            x_g = []
            for g in range(KI // JG):
                xt = xpool.tile([128, JG * BC], BF16, name="x_g", tag="x_g")
                nc.sync.dma_start(xt[:], xP[:, g * JG * BC:(g + 1) * JG * BC])
                x_g.append(xt)

            # ---- stage A: paired matmuls into 2-bank PSUM ----
            s_lo = spool.tile([96, JH * BC], BF16, name="s_lo", tag="s_lo")
            s_hi = spool.tile([96, JH * BC], BF16, name="s_hi", tag="s_hi")
            for p2 in range(KI // 2):
                ps = psum.tile([96, 2 * BC], mybir.dt.float32,
                               name="ps_a", tag="ps")
                for k in range(2):
                    j = 2 * p2 + k
                    rhs = x_g[j // JG][:, (j % JG) * BC:(j % JG + 1) * BC]
                    nc.tensor.matmul(ps[:, k * BC:(k + 1) * BC], f_t[:],
                                     rhs, start=True, stop=True)
                if p2 < KI // 4:
                    nc.vector.tensor_copy(
                        s_lo[:, 2 * p2 * BC:(2 * p2 + 2) * BC], ps[:])
                else:
                    c0 = (2 * p2 - JH) * BC
                    nc.scalar.copy(s_hi[:, c0:c0 + 2 * BC], ps[:])

            # ---- regroup 1 writes: (flp, j-half) -> DRAM, 8 DMAs ----
            for flp in range(4):
                for hh, sh in enumerate((s_lo, s_hi)):
                    nc.sync.dma_start(
                        sS_w[flp, :, hh * JH:(hh + 1) * JH],
                        sh[flp * 24:(flp + 1) * 24, :].rearrange(
                            "e (j b) -> e j b", j=JH))

            # ---- regroup 1 reads: plain column slices ----
            g_q = []
            for qd in range(NP // EQ):
                gq = gpool.tile([128, EQ * BC], BF16, name="g_q", tag="g_q")
                nc.sync.dma_start(
                    gq[:], sS_g[:, qd * EQ * BC:(qd + 1) * EQ * BC])
                g_q.append(gq)

            # ---- stage M: paired matmuls, mid rows (fl q i) ----
            m_lo = mpool.tile([128, EH * BC], BF16, name="m_lo", tag="m_lo")
            m_hi = mpool.tile([128, EH * BC], BF16, name="m_hi", tag="m_hi")
            for p2 in range(NP // 2):
                ps = psum.tile([128, 2 * BC], mybir.dt.float32,
                               name="ps_m", tag="ps")
                for k in range(2):
                    e = 2 * p2 + k
                    rhs = g_q[e // EQ][:, (e % EQ) * BC:(e % EQ + 1) * BC]
                    nc.tensor.matmul(ps[:, k * BC:(k + 1) * BC],
                                     w_all[:, e * 128:(e + 1) * 128], rhs,
                                     start=True, stop=True)
                if p2 < NP // 4:
                    nc.vector.tensor_copy(
                        m_lo[:, 2 * p2 * BC:(2 * p2 + 2) * BC], ps[:])
                else:
                    c0 = (2 * p2 - EH) * BC
                    nc.scalar.copy(m_hi[:, c0:c0 + 2 * BC], ps[:])

            # ---- regroup 2 writes: (flq, e-half) -> DRAM, 8 DMAs ----
            for flq in range(4):
                for hh, mh in enumerate((m_lo, m_hi)):
                    nc.scalar.dma_start(
                        sM_w[flq, :, hh * EH:(hh + 1) * EH],
                        mh[flq * 32:(flq + 1) * 32, :].rearrange(
                            "i (e b) -> i e b", e=EH))

            # ---- regroup 2 reads: plain column slices ----
            h_q = []
            for qd in range(KO // IQ):
                hq = hpool.tile([96, IQ * BC], BF16, name="h_q", tag="h_q")
                nc.scalar.dma_start(
                    hq[:], sM_h[:, qd * IQ * BC:(qd + 1) * IQ * BC])
                h_q.append(hq)

            # ---- stage C: paired matmuls -> out groups ----
            o_g = None
            for p2 in range(KO // 2):
                if (2 * p2) % JG == 0:
                    o_g = opool.tile([128, JG * BC], BF16, name="o_g",
                                     tag="o_g")
                ps = psum.tile([128, 2 * BC], mybir.dt.float32,
                               name="ps_c", tag="ps")
                for k in range(2):
                    i = 2 * p2 + k
                    rhs = h_q[i // IQ][:, (i % IQ) * BC:(i % IQ + 1) * BC]
                    nc.tensor.matmul(ps[:, k * BC:(k + 1) * BC], g_t[:],
                                     rhs, start=True, stop=True)
                c0 = ((2 * p2) % JG) * BC
                if ((2 * p2) // JG) % 2 == 0:
                    nc.vector.tensor_copy(o_g[:, c0:c0 + 2 * BC], ps[:])
                else:
                    nc.scalar.copy(o_g[:, c0:c0 + 2 * BC], ps[:])
                if (2 * p2 + 2) % JG == 0:
                    col0 = (2 * p2 + 2 - JG) * BC
                    nc.sync.dma_start(oP[:, col0:col0 + JG * BC], o_g[:])
    nc.finalize()
    return nc


def _get_nc():
    if "nc" not in _CACHE:
        _CACHE["nc"] = _build_nc()
    return _CACHE["nc"]


def _host_weights(W_real, W_imag):
    """F [128,96], G [96,128], Wmid [24,128,128] (float64)."""
    t = np.arange(B)[:, None].astype(np.float64)
    # F columns ordered (fl, p, e): f = 2e + fl; p=0 -> cos, p=1 -> -sin.
    F = np.zeros((128, 96))
    for fl in range(2):
        for p in range(2):
            for e in range(NP):
                f = 2 * e + fl
                col = fl * 48 + p * 24 + e
                w = 2 * np.pi * f * t[:, 0] / B
                F[:, col] = np.cos(w) if p == 0 else -np.sin(w)
    # G rows ordered (fl, q, e) with f = 2e + fl: q=0 -> scale*cos,
    # q=1 -> -scale*sin
    G = np.zeros((96, 128))
    scale = np.full(KT, 2.0 / B)
    scale[0] = 1.0 / B
    for fl in range(2):
        for q in range(2):
            for e in range(NP):
                f = 2 * e + fl
                w = 2 * np.pi * f * np.arange(B) / B
                G[fl * 48 + q * 24 + e] = (scale[f] * np.cos(w) if q == 0
                                           else -scale[f] * np.sin(w))
    # Wmid[e]: rows (fl, p, j), cols (fl, q, i); block-diag in fl
    Wr = W_real.astype(np.float64)
    Wi = W_imag.astype(np.float64)
    Wm = np.zeros((NP, 128, 128))
    for e in range(NP):
        for fl in range(2):
            f = 2 * e + fl
            r0, c0 = fl * 64, fl * 64
            # q=0: Re_out = Wr @ Re + Wi @ Im ; q=1: Im_out = Wr @ Im - Wi @ Re
            Wrf = Wr[:, :, f].T  # [j, i]
            Wif = Wi[:, :, f].T
            Wm[e, r0:r0 + 32, c0:c0 + 32] = Wrf            # p0 -> q0: Wr
            Wm[e, r0 + 32:r0 + 64, c0:c0 + 32] = Wif       # p1 -> q0: Wi
            Wm[e, r0:r0 + 32, c0 + 32:c0 + 64] = -Wif      # p0 -> q1: -Wi
            Wm[e, r0 + 32:r0 + 64, c0 + 32:c0 + 64] = Wrf  # p1 -> q1: Wr
    return F, G, Wm


def kernel(x, W_real, W_imag):
    global LAST_RESULTS
    import ml_dtypes
    from concourse.bass_utils import run_bass_kernel_spmd

    bf16 = np.dtype(ml_dtypes.bfloat16)
    x = np.asarray(x, dtype=np.float32)
    F, G, Wm = _host_weights(np.asarray(W_real), np.asarray(W_imag))
    Fb = np.ascontiguousarray(F).astype(bf16)
    Gb = np.ascontiguousarray(G).astype(bf16)
    wm_packed = np.ascontiguousarray(
        Wm.transpose(1, 0, 2).reshape(128, NP * 128)).astype(bf16)

    # pack x: [batch, (j t)] -> per core [t, (j, b)]
    xr = x.reshape(N_CORES, BC, KI, B).transpose(0, 3, 2, 1)
    xr = np.ascontiguousarray(xr.reshape(N_CORES, B, KI * BC)).astype(bf16)

    in_maps = []
    for core in range(N_CORES):
        in_maps.append({"xP": xr[core], "fmat": Fb, "gmat": Gb,
                        "wmid": wm_packed})

    nc = _get_nc()
    res = run_bass_kernel_spmd(nc, in_maps, list(range(N_CORES)), trace=TRACE)
    LAST_RESULTS = res

    out = np.empty((BATCH, OUT_F), np.float32)
    for core in range(N_CORES):
        oPc = np.asarray(res.results[core]["oP"]).astype(np.float32)
        # [t, (i, b)] -> [b, (i t)]
        oc = oPc.reshape(B, KO, BC).transpose(2, 1, 0)
        out[core * BQ:(core + 1) * BQ, :] = oc.reshape(BQ, OUT_F)
    return out
